# revision 7
# baseline (speedup 1.0000x reference)
"""MHSA + RoPE kernel for Trainium2, 8 NeuronCores.

Sharding: data-parallel over batch (B=2) x tensor-parallel over heads
(16 heads -> 4 head-groups of 4). Core c handles batch c//4, heads
[4*(c%4) : 4*(c%4)+4]. Each core computes its partial o_proj output
[N, D]; host sums the 4 partials per batch (the "all-reduce").

Device-side layout choices (per core):
  - q/k are computed directly in transposed layout qT/kT [d, n] so the
    scores matmul S^T[j,i] = k_j . q_i needs no transposes, and the PV
    matmul produces A^T [d, i] which is exactly the lhsT layout o_proj
    needs.
  - softmax denominators: S^T tiles are exp'd on ScalarE, accumulated
    elementwise over j-tiles on VectorE, then partition-reduced with a
    ones-vector matmul; reciprocal is broadcast back over partitions
    with a ones-row matmul.
  - projections and o_proj run in bf16 (inputs pre-cast on host);
    scores / PV run in float32r (full-rate fp32 mode of the PE).
"""

import sys

sys.path.insert(0, "/opt/trn_rl_repo")

import numpy as np
import ml_dtypes

import concourse.bass as bass
import concourse.tile as tile
from concourse import bacc, mybir
from concourse.bass_utils import run_bass_kernel_spmd

F32 = mybir.dt.float32
F32R = mybir.dt.float32r
BF16 = mybir.dt.bfloat16
MULT = mybir.AluOpType.mult
ADD = mybir.AluOpType.add
EXP = mybir.ActivationFunctionType.Exp
PSUM = bass.MemorySpace.PSUM

B, N, D = 2, 2048, 2048
H, HD = 16, 128
HL = 4            # local heads per core
C = HL * HD       # 512 local head cols
KT = D // 128     # 16 contraction tiles
NB = 4            # n-blocks of 512 for projections
NT = N // 128     # 16 j-tiles
SCALE = float(HD) ** -0.5
N_CORES = 8

_CACHE = {}


def _build_program():
    nc = bacc.Bacc("TRN2", target_bir_lowering=False, debug=False,
                   num_devices=N_CORES)

    xt_d = nc.dram_tensor("xt", [NB, 128, KT, 512], BF16, kind="ExternalInput")
    wq_d = nc.dram_tensor("wq", [128, KT, C], BF16, kind="ExternalInput")
    wk_d = nc.dram_tensor("wk", [128, KT, C], BF16, kind="ExternalInput")
    wv_d = nc.dram_tensor("wv", [128, KT, C], BF16, kind="ExternalInput")
    wo_d = nc.dram_tensor("wo", [128, HL, D], BF16, kind="ExternalInput")
    cos_d = nc.dram_tensor("cos", [128, N], F32R, kind="ExternalInput")
    sin_d = nc.dram_tensor("sin", [128, N], F32R, kind="ExternalInput")
    onec_d = nc.dram_tensor("onec", [128, 1], F32R, kind="ExternalInput")
    oner_d = nc.dram_tensor("oner", [1, 128], F32R, kind="ExternalInput")
    out_d = nc.dram_tensor("out", [N, D], F32, kind="ExternalOutput")

    with tile.TileContext(nc) as tc:
        with tc.tile_pool(name="res", bufs=1) as res:
            qr = res.tile([128, HL, N], F32R)    # q^T per head [d, n]
            kr = res.tile([128, HL, N], F32R)    # k^T per head [d, n]
            vv = res.tile([128, NT, C], F32R)    # v natural [n, c]
            ones_col = res.tile([128, 1], F32R)
            ones_row = res.tile([1, 128], F32R)
            nc.sync.dma_start(ones_col[:], onec_d[:])
            nc.sync.dma_start(ones_row[:], oner_d[:])

            # ---------------- Phase 1: Q/K/V projections (bf16) ---------
            with (
                tc.tile_pool(name="p1", bufs=1) as p1,
                tc.tile_pool(name="ps1", bufs=6, space=PSUM) as ps1,
            ):
                w_sbs = []
                for wd, wname in ((wq_d, "wq"), (wk_d, "wk"), (wv_d, "wv")):
                    w_sb = p1.tile([128, KT, C], BF16, tag=f"w_{wname}")
                    nc.sync.dma_start(w_sb[:], wd[:])
                    w_sbs.append(w_sb)

                for nb in range(NB):
                    x_sb = p1.tile([128, KT, 512], BF16, tag="x")
                    nc.sync.dma_start(x_sb[:], xt_d[nb])
                    nsl = bass.ts(nb, 512)
                    # q and k projections: psum [d(128) x n(512)] per head
                    for pi, (w_sb, dst) in enumerate(
                        ((w_sbs[0], qr), (w_sbs[1], kr))
                    ):
                        for m in range(HL):
                            ps = ps1.tile([128, 512], F32, tag="ps")
                            for t in range(KT):
                                nc.tensor.matmul(
                                    ps[:],
                                    w_sb[:, t, bass.ts(m, 128)],
                                    x_sb[:, t, :],
                                    start=(t == 0),
                                    stop=(t == KT - 1),
                                )
                            nc.scalar.copy(dst[:, m, nsl], ps[:])
                    # v projection: psum [n(128) x c(512)] per n-chunk
                    for m in range(HL):
                        ps = ps1.tile([128, 512], F32, tag="ps")
                        for t in range(KT):
                            nc.tensor.matmul(
                                ps[:],
                                x_sb[:, t, bass.ts(m, 128)],
                                w_sbs[2][:, t, :],
                                start=(t == 0),
                                stop=(t == KT - 1),
                            )
                        nc.scalar.copy(vv[:, nb * HL + m, :], ps[:])

            # ---------------- Phase 2: RoPE + attention -----------------
            with tc.tile_pool(name="aop", bufs=1) as aop:
                ao = aop.tile([128, HL, N], BF16)   # A^T normalized [c, n]

                with (
                    tc.tile_pool(name="p2", bufs=1) as p2,
                    tc.tile_pool(name="ps_s", bufs=2, space=PSUM) as ps_s,
                    tc.tile_pool(name="ps_a", bufs=1, space=PSUM) as ps_a,
                ):
                    cos_sb = p2.tile([128, N], F32R, tag="cos")
                    sin_sb = p2.tile([128, N], F32R, tag="sin")
                    nc.sync.dma_start(cos_sb[:], cos_d[:])
                    nc.sync.dma_start(sin_sb[:], sin_d[:])

                    # RoPE, in place:  t = shift(q) * sin_signed;
                    # q *= cos; q += t   (sign of sin folded in on host).
                    # The d-half swap is a partition shuffle - compute
                    # engines can't shift partitions, so do it with an
                    # SBUF->SBUF DMA.
                    for src in (qr, kr):
                        for h in range(HL):
                            sl = src[:, h, :]
                            tmp = p2.tile([128, N], F32R, tag="tmp")
                            nc.sync.dma_start(tmp[0:64, :], sl[64:128, :])
                            nc.sync.dma_start(tmp[64:128, :], sl[0:64, :])
                            nc.vector.tensor_tensor(tmp[:], tmp[:], sin_sb[:],
                                                    op=MULT)
                            nc.vector.tensor_tensor(sl, sl, cos_sb[:], op=MULT)
                            nc.vector.tensor_tensor(sl, sl, tmp[:], op=ADD)

                    for h in range(HL):
                        a_ps = ps_a.tile([128, N], F32, tag="a")
                        acc = p2.tile([128, N], F32R, tag="acc")
                        for ih in range(2):
                            ihsl = bass.ts(ih, 1024)
                            for j in range(NT):
                                s_ps = ps_s.tile([128, 1024], F32, tag="s")
                                for f in range(2):
                                    nc.tensor.matmul(
                                        s_ps[:, bass.ts(f, 512)],
                                        kr[:, h, bass.ts(j, 128)],
                                        qr[:, h, ih * 1024 + f * 512 : ih * 1024 + (f + 1) * 512],
                                        start=True, stop=True,
                                    )
                                s_exp = p2.tile([128, 1024], F32R, tag="sexp")
                                nc.scalar.activation(s_exp[:], s_ps[:], EXP,
                                                     scale=SCALE)
                                if j == 0:
                                    nc.vector.tensor_copy(acc[:, ihsl], s_exp[:])
                                else:
                                    nc.vector.tensor_tensor(
                                        acc[:, ihsl], acc[:, ihsl], s_exp[:],
                                        op=ADD)
                                for f in range(2):
                                    nc.tensor.matmul(
                                        a_ps[:, ih * 1024 + f * 512 : ih * 1024 + (f + 1) * 512],
                                        vv[:, j, bass.ts(h, 128)],
                                        s_exp[:, bass.ts(f, 512)],
                                        start=(j == 0), stop=(j == NT - 1),
                                    )
                        # softmax denominators: partition-reduce acc with a
                        # ones matmul, reciprocal, broadcast back over
                        # partitions with a ones-row matmul
                        recip = p2.tile([1, N], F32R, tag="recip")
                        for ih in range(2):
                            l_ps = ps_s.tile([1, 1024], F32, tag="s")
                            for f in range(2):
                                nc.tensor.matmul(
                                    l_ps[:, bass.ts(f, 512)],
                                    ones_col[:],
                                    acc[:, ih * 1024 + f * 512 : ih * 1024 + (f + 1) * 512],
                                    start=True, stop=True,
                                )
                            with nc.allow_low_precision(
                                reason="f32r rounding of softmax recip "
                                       "denominators is ~2^-19 relative"):
                                nc.vector.reciprocal(
                                    recip[:, bass.ts(ih, 1024)], l_ps[:])
                        bc_sb = p2.tile([128, N], F32, tag="bcsb")
                        for ih in range(2):
                            bc_ps = ps_s.tile([128, 1024], F32, tag="s")
                            for f in range(2):
                                nc.tensor.matmul(
                                    bc_ps[:, bass.ts(f, 512)],
                                    ones_row[:],
                                    recip[0:1, ih * 1024 + f * 512 : ih * 1024 + (f + 1) * 512],
                                    start=True, stop=True,
                                )
                            nc.scalar.copy(bc_sb[:, bass.ts(ih, 1024)], bc_ps[:])
                        nc.vector.tensor_tensor(ao[:, h, :], a_ps[:], bc_sb[:],
                                                op=MULT)

                # ---------------- Phase 3: o_proj (bf16) ----------------
                with (
                    tc.tile_pool(name="p3", bufs=1) as p3,
                    tc.tile_pool(name="ps3", bufs=4, space=PSUM) as ps3,
                ):
                    wo_sb = p3.tile([128, HL, D], BF16, tag="wo")
                    nc.sync.dma_start(wo_sb[:], wo_d[:])
                    for m in range(NT):
                        st = p3.tile([128, D], F32, tag="st")
                        for f in range(4):
                            o_ps = ps3.tile([128, 512], F32, tag="o")
                            for ct in range(HL):
                                nc.tensor.matmul(
                                    o_ps[:],
                                    ao[:, ct, bass.ts(m, 128)],
                                    wo_sb[:, ct, bass.ts(f, 512)],
                                    start=(ct == 0), stop=(ct == HL - 1),
                                )
                            nc.scalar.copy(st[:, bass.ts(f, 512)], o_ps[:])
                        nc.sync.dma_start(out_d[bass.ts(m, 128), :], st[:])

    nc.compile()
    return nc


def _rope_tables():
    inv_freq = 1.0 / (10000.0 ** (np.arange(0, HD, 2, dtype=np.float32) / HD))
    pos = np.arange(N, dtype=np.float32)
    freqs = pos[:, None] * inv_freq[None, :]          # [N, HD/2]
    emb = np.concatenate([freqs, freqs], axis=-1)     # [N, HD]
    cos = np.cos(emb).astype(np.float32).T.copy()     # [HD, N]
    sin = np.sin(emb).astype(np.float32).T.copy()     # [HD, N]
    sin_signed = sin.copy()
    sin_signed[0:64] *= -1.0
    return cos, sin_signed


def _make_in_maps(x, Wq, Wk, Wv, Wo):
    cos, sin_signed = _rope_tables()
    bf = ml_dtypes.bfloat16

    in_maps = []
    for c in range(N_CORES):
        b, hg = c // 4, c % 4
        cols = slice(C * hg, C * hg + C)
        xT = np.ascontiguousarray(x[b].T)                      # [D, N]
        xt = np.ascontiguousarray(
            xT.reshape(KT, 128, NB, 512).transpose(2, 1, 0, 3)
        ).astype(bf)                                           # [NB,128,KT,512]

        def wslice(W):
            wt = W[cols, :].T                                  # [D, C]
            return np.ascontiguousarray(
                wt.reshape(KT, 128, C).transpose(1, 0, 2)
            ).astype(bf)                                       # [128, KT, C]

        wo_t = Wo[:, cols].T                                   # [C, D]
        wo = np.ascontiguousarray(
            wo_t.reshape(HL, 128, D).transpose(1, 0, 2)
        ).astype(bf)                                           # [128, HL, D]

        in_maps.append({
            "xt": xt,
            "wq": wslice(Wq),
            "wk": wslice(Wk),
            "wv": wslice(Wv),
            "wo": wo,
            "cos": cos,
            "sin": sin_signed,
            "onec": np.ones((128, 1), dtype=np.float32),
            "oner": np.ones((1, 128), dtype=np.float32),
        })
    return in_maps


def kernel(x, Wq, Wk, Wv, Wo):
    x = np.asarray(x, dtype=np.float32)
    Wq = np.asarray(Wq, dtype=np.float32)
    Wk = np.asarray(Wk, dtype=np.float32)
    Wv = np.asarray(Wv, dtype=np.float32)
    Wo = np.asarray(Wo, dtype=np.float32)

    if "nc" not in _CACHE:
        _CACHE["nc"] = _build_program()
    nc = _CACHE["nc"]

    in_maps = _make_in_maps(x, Wq, Wk, Wv, Wo)
    results = run_bass_kernel_spmd(
        nc, in_maps, core_ids=list(range(N_CORES))
    ).results

    out = np.zeros((B, N, D), dtype=np.float32)
    for c in range(N_CORES):
        out[c // 4] += results[c]["out"]
    return out
